# revision 23
# baseline (speedup 1.0000x reference)
"""Trainium2 Bass kernel for nn_AttentionPropagation (sparse attention propagation).

Reference computation:
  Q = cat(dense_xyz, dense_feat) @ Wq.T + bq            [B, N2, F]
  K = cat(sparse_xyz, sparse_feat) @ Wk.T + bk          [B, N1, F]
  V = sparse_feat @ Wv.T + bv                           [B, N1, F]
  attn = softmax(Q K^T / sqrt(F) - 0.5 * dist(dense_xyz, sparse_xyz))
  out = (attn @ V + dense_feat) @ Wo.T + bo             [B, N2, F]

Shapes: B=2, N1=4096 (sparse/keys), N2=32768 (dense/queries), F=128.
Sharding: queries (N2) split across 8 cores; sparse K/V + weights replicated.

Per-core kernel, transposed layout (keys on partitions, queries on free dim).
Per pair of key chunks (2x128 keys x 512 queries, one [128,1024] PSUM tile):
  ds = 0.25*dist^2            (PE, K=16 fp16 hi/lo-split aug matmul; exact)
  tile = sqrt(ds + eps)       (ACT, in place on PSUM)
  tile += -st                 (PE accumulates -K@Q^T; K negated on host)
  spre = copy(tile) -> f16    (DVE; spre = 0.5*dist - st)
  attn = exp(-spre) -> fp8e4  (ACT, scale=-1)
  P^T += V8 @ attn            (PE, fp8 DoubleRow over chunk pairs; V in
                               single-level fp8e4 -- softmax renormalization
                               cancels most of the quantization)
  sums += ones @ attn         (PE, fp8 DoubleRow, all-ones stationary ->
                               every psum row holds the sums; no broadcast)
  out^T = Wo @ ((P^T * 1/sums) + dense_feat^T) + bo'   (bo' = Wo@bv + bo,
                               folded on host; the xt add runs on Pool)
The old standalone DVE subtract pass and the Pool partition_broadcast are
gone; sqrt and exp still alternate in table-set phases over units of 2-3
query groups to amortize the ~1.3us ACT table loads.  Phase A is
software-pipelined (ds of pair i+1 ahead of st of pair i) and each unit's
phase B pre-issues the next unit's DMAs, Q projection and first ds pairs.
"""

import os
import numpy as np

os.environ.setdefault("JAX_COMPILATION_CACHE_DIR", "/tmp/jax_bass_cache")
os.environ.setdefault("JAX_PERSISTENT_CACHE_MIN_ENTRY_SIZE_BYTES", "0")
os.environ.setdefault("JAX_PERSISTENT_CACHE_MIN_COMPILE_TIME_SECS", "1")

import concourse.bacc as bacc
import concourse.tile as tile
import concourse.mybir as mybir
from concourse import bass_utils
from concourse.tile import add_dep_helper

F32 = mybir.dt.float32
F32R = mybir.dt.float32r
F16 = mybir.dt.float16
F8 = mybir.dt.float8e4
AF = mybir.ActivationFunctionType
OP = mybir.AluOpType
DR = mybir.MatmulPerfMode.DoubleRow

B = 2
N1 = 4096          # sparse points (keys)
N2 = 32768         # dense points (queries)
FEAT = 128
SCALE = FEAT ** -0.5
NCORES = 8
QPC = N2 // NCORES  # queries per core per batch (4096)
QG = 512            # query group (matmul moving free dim)
GROUPS = QPC // QG  # 8 groups per batch
KC = 128            # key chunk (PSUM partition dim)
CHUNKS = N1 // KC   # 32
PAIRS = CHUNKS // 2  # 16 chunk pairs (one [128,1024] psum tile each)
UNITS = [(0, 1), (2, 3, 4), (5, 6, 7)]  # groups per sqrt/exp table unit
NAUG = 16           # hi/lo-split dist^2 augmentation rows
SQRT_EPS = 1e-5     # covers residual fp16-split ds error (~3e-6) and the
                    # reference's 1e-12 clamp; adds <2e-3 logit error

_NC_CACHE = {}


def _r2(ap):
    # [p, (two f)] -> [p, two, f] for DoubleRow matmuls
    return ap.rearrange("p (two f) -> p two f", two=2)


def _build():
    if "nc" in _NC_CACHE:
        return _NC_CACHE["nc"]
    nc = bacc.Bacc("TRN2", target_bir_lowering=False, debug=False)

    # ---- DRAM I/O (per-core shard) ----
    dfT = nc.dram_tensor("dfT", [B, FEAT, QPC], F32R, kind="ExternalInput")
    dxT = nc.dram_tensor("dxT", [B, 3, QPC], F32R, kind="ExternalInput")
    qaug = nc.dram_tensor("qaug", [B, NAUG, QPC], F16, kind="ExternalInput")
    sfT16 = nc.dram_tensor("sfT16", [B, FEAT, N1], F16, kind="ExternalInput")
    kxT = nc.dram_tensor("kxT", [B, FEAT, N1], F16, kind="ExternalInput")  # -(xyz@WkxT+bk)
    kaug = nc.dram_tensor("kaug", [B, NAUG, N1], F16, kind="ExternalInput")
    WqfT = nc.dram_tensor("WqfT", [FEAT, FEAT], F32R, kind="ExternalInput")
    WqxT = nc.dram_tensor("WqxT", [3, FEAT], F32R, kind="ExternalInput")
    WkfT = nc.dram_tensor("WkfT", [FEAT, FEAT], F16, kind="ExternalInput")  # negated
    WvT = nc.dram_tensor("WvT", [FEAT, FEAT], F16, kind="ExternalInput")
    WoT = nc.dram_tensor("WoT", [FEAT, FEAT], F16, kind="ExternalInput")
    bq = nc.dram_tensor("bq", [FEAT, 1], F32, kind="ExternalInput")
    bv = nc.dram_tensor("bv", [FEAT, 1], F32, kind="ExternalInput")
    bo = nc.dram_tensor("bo", [FEAT, 1], F32, kind="ExternalInput")
    outT = nc.dram_tensor("outT", [B, FEAT, QPC], F32, kind="ExternalOutput")

    with tile.TileContext(nc) as tc:
        with tc.tile_pool(name="const", bufs=1) as const_p, \
             tc.tile_pool(name="batch", bufs=1) as batch_p, \
             tc.tile_pool(name="slab", bufs=3) as slab_p, \
             tc.tile_pool(name="attn", bufs=2) as attn_p, \
             tc.tile_pool(name="stage", bufs=1) as stage_p, \
             tc.tile_pool(name="gsmall", bufs=4) as gsm_p, \
             tc.tile_pool(name="gout", bufs=2) as gout_p, \
             tc.tile_pool(name="ps_pair", bufs=3, space="PSUM") as ps_pair, \
             tc.tile_pool(name="ps_pt", bufs=1, space="PSUM") as ps_pt, \
             tc.tile_pool(name="ps_sm", bufs=1, space="PSUM") as ps_sm:

            # ---- constants ----
            wqf_t = const_p.tile([FEAT, FEAT], F32R)
            wqx_t = const_p.tile([3, FEAT], F32R)
            wkf_t = const_p.tile([FEAT, FEAT], F16)
            wv_t = const_p.tile([FEAT, FEAT], F16)
            wo_t = const_p.tile([FEAT, FEAT], F16)
            bq_t = const_p.tile([FEAT, 1], F32)
            bv_t = const_p.tile([FEAT, 1], F32)
            bo_t = const_p.tile([FEAT, 1], F32)
            ones8 = const_p.tile([KC, 2 * KC], F8)
            eps_t = const_p.tile([KC, 1], F32)
            dummy_t = const_p.tile([1, 1], F32)
            nc.vector.memset(eps_t, SQRT_EPS)
            # preload the sqrt table at t=0, overlapping the input DMAs
            nc.scalar.activation(dummy_t, eps_t[0:1, 0:1], AF.Sqrt,
                                 bias=eps_t[0:1, 0:1])
            for t, d in ((wqf_t, WqfT), (wqx_t, WqxT), (wkf_t, WkfT),
                         (wv_t, WvT), (wo_t, WoT),
                         (bq_t, bq), (bv_t, bv), (bo_t, bo)):
                # constants ride the SWDGE queue to keep the HWDGE queue free
                # for the first group's inputs at startup
                nc.gpsimd.dma_start(out=t, in_=d.ap())
            nc.vector.memset(ones8, 1.0)

            # ACT runs exactly two table-based funcs (Sqrt, Exp) in different
            # table sets; a table switch costs ~1.3us.  Pin every sqrt of
            # unit u+1 behind the last exp of unit u so the scheduler cannot
            # interleave the phases.
            last_exp = [None]
            last_sqrt = [None]

            bst = {}      # batch -> dict of per-batch slabs
            pre_dma = {}  # (b, g) -> (df_t, dx_t, qa_t)
            pre_qt = {}   # (b, g) -> qt tile
            pre_ds = {}   # (b, g, p) -> pair psum tile (ds done, sqrt pending)

            def batch_dmas(b):
                st = {}
                st["sf16"] = stage_p.tile([FEAT, N1], F16, tag="sf16", name="sf16")
                st["kx"] = stage_p.tile([FEAT, N1], F16, tag="kx", name="kx")
                st["ka"] = batch_p.tile([NAUG, N1], F16, tag="ka", name="ka")
                nc.sync.dma_start(out=st["ka"], in_=kaug.ap()[b])
                bst[b] = st
                return st

            def batch_dmas2(b):
                st = bst[b]
                nc.sync.dma_start(out=st["kx"], in_=kxT.ap()[b])
                # chunked so the j=0 K^T projection starts on the first slice
                for j in range(N1 // QG):
                    nc.sync.dma_start(
                        out=st["sf16"][:, j * QG:(j + 1) * QG],
                        in_=sfT16.ap()[b, :, j * QG:(j + 1) * QG])

            def kproj_alloc(b):
                st = bst[b]
                st["kt"] = batch_p.tile([FEAT, N1], F16, tag="kt", name="kt")  # -K^T
                st["vhi"] = batch_p.tile([KC, CHUNKS * FEAT], F8, tag="vhi", name="vhi")
                st["kproj_j"] = 0

            def kproj_step(b, upto_j):
                st = bst[b]
                while st["kproj_j"] <= min(upto_j, N1 // QG - 1):
                    j = st["kproj_j"]
                    ps = ps_pair.tile([KC, 2 * QG], F32, tag="pair")
                    nc.tensor.matmul(ps[:, 0:QG], wkf_t,
                                     st["sf16"][:, j * QG:(j + 1) * QG],
                                     start=True, stop=True)
                    # xyz part of -K plus bias, folded on the host into kxT
                    nc.vector.tensor_tensor(
                        out=st["kt"][:, j * QG:(j + 1) * QG], in0=ps[:, 0:QG],
                        in1=st["kx"][:, j * QG:(j + 1) * QG], op=OP.add)
                    st["kproj_j"] = j + 1

            def batch_kproj(b):
                kproj_alloc(b)
                kproj_step(b, N1 // QG - 1)

            def project_v(b):
                # V is only read in phase B; emitting it after the first
                # unit's phase A keeps PE on the critical path early on.
                # Single-level fp8 V: softmax renormalization cancels most
                # of the quantization (golden model: +1e-4 rel err).
                st = bst[b]
                for c in range(CHUNKS):
                    ps = ps_pt.tile([KC, QG], F32, tag="pt")
                    nc.tensor.matmul(ps[:, 0:FEAT],
                                     st["sf16"][:, c * KC:(c + 1) * KC], wv_t,
                                     start=True, stop=True)
                    nc.vector.tensor_copy(
                        st["vhi"][:, c * FEAT:(c + 1) * FEAT], ps[:, 0:FEAT])

            def group_dmas(b, g):
                q0 = g * QG
                df_t = gsm_p.tile([FEAT, QG], F32R, tag="df", bufs=4)
                dx_t = gsm_p.tile([3, QG], F32R, tag="dx")
                qa_t = gsm_p.tile([NAUG, QG], F16, tag="qa")
                # qa first: it gates the ds matmuls -> sqrt pipeline
                nc.sync.dma_start(out=qa_t, in_=qaug.ap()[b, :, q0:q0 + QG])
                nc.sync.dma_start(out=dx_t, in_=dxT.ap()[b, :, q0:q0 + QG])
                nc.sync.dma_start(out=df_t, in_=dfT.ap()[b, :, q0:q0 + QG])
                r = (df_t, dx_t, qa_t)
                pre_dma[(b, g)] = r
                return r

            def do_qt(b, g):
                df_t, dx_t, qa_t = pre_dma[(b, g)]
                ps_q = ps_pair.tile([KC, 2 * QG], F32, tag="pair")
                nc.tensor.matmul(ps_q[:, 0:QG], wqf_t, df_t,
                                 start=True, stop=False)
                nc.tensor.matmul(ps_q[:, 0:QG], wqx_t, dx_t,
                                 start=False, stop=True)
                qt_t = gsm_p.tile([FEAT, QG], F16, tag="qt")
                nc.vector.tensor_scalar_add(qt_t, ps_q[:, 0:QG], bq_t)
                pre_qt[(b, g)] = qt_t
                return qt_t

            def pair_ds(b, g, p):
                """ds matmuls into a fresh pair tile (PE only)."""
                qa_t = pre_dma[(b, g)][2]
                ka_t = bst[b]["ka"]
                pt_ps = ps_pair.tile([KC, 2 * QG], F32, tag="pair")
                for ci, c in ((0, 2 * p), (1, 2 * p + 1)):
                    nc.tensor.matmul(pt_ps[:, ci * QG:(ci + 1) * QG],
                                     ka_t[:, c * KC:(c + 1) * KC],
                                     qa_t, start=True, stop=True)
                pre_ds[(b, g, p)] = pt_ps
                return pt_ps

            def pair_sqrt(pt_ps):
                sq_i = nc.scalar.activation(pt_ps, pt_ps, AF.Sqrt,
                                            bias=eps_t[:, 0:1])
                if last_exp[0] is not None:
                    add_dep_helper(sq_i.ins, last_exp[0],
                                   reason="ACT table phase order")
                last_sqrt[0] = sq_i.ins

            def pair_back(b, g, p, pt_ps, sp):
                """-st accumulate onto the sqrt'd tile + f16 copy out."""
                qt_t = pre_qt[(b, g)]
                kt_t = bst[b]["kt"]
                c0 = 2 * p
                for ci, c in ((0, c0), (1, c0 + 1)):
                    nc.tensor.matmul(pt_ps[:, ci * QG:(ci + 1) * QG],
                                     kt_t[:, c * KC:(c + 1) * KC], qt_t,
                                     start=False, stop=True,
                                     skip_group_check=True)
                nc.vector.tensor_copy(sp[:, c0 * QG:(c0 + 2) * QG], pt_ps)

            units_all = [(b, gs) for b in range(B) for gs in UNITS]
            NPRE = 2  # pair tiles pre-ds'ed for the next unit during phase B

            # ---- startup: first group's DMAs + Q proj, then batch-0 setup ----
            batch_dmas(0)
            group_dmas(0, 0)
            pair_ds(0, 0, 0)
            pair_ds(0, 0, 1)
            do_qt(0, 0)
            batch_dmas2(0)
            batch_kproj(0)

            for ui, (b, gs) in enumerate(units_all):
                first_of_batch = (gs is UNITS[0])
                nxt = units_all[ui + 1] if ui + 1 < len(units_all) else None

                # ---------- phase A: ds -> sqrt -> -st -> f16 copy ----------
                # Software-pipelined: the ds+sqrt of pair i+1 is emitted
                # before the st matmuls of pair i, so the in-order PE queue
                # never waits on the ACT sqrt.
                spre = {}
                work = []  # (b, g, p, pt_ps, sp) awaiting back-half
                for g in gs:
                    if (b, g) not in pre_dma:
                        group_dmas(b, g)
                    if (b, g) not in pre_qt:
                        do_qt(b, g)
                    sp = slab_p.tile([KC, CHUNKS * QG], F16, tag="spre")
                    spre[g] = sp
                    for p in range(PAIRS):
                        pt_ps = pre_ds.pop((b, g, p), None)
                        if pt_ps is None:
                            pt_ps = pair_ds(b, g, p)
                        pair_sqrt(pt_ps)
                        work.append((b, g, p, pt_ps, sp))
                        if len(work) > 1:
                            pair_back(*work.pop(0))
                while work:
                    pair_back(*work.pop(0))

                if first_of_batch:
                    project_v(b)

                # ---------- phase B: exp -> attn@V (fp8 DR) -> out ----------
                for gi, g in enumerate(gs):
                    q0 = g * QG
                    sp = spre[g]
                    ap8 = attn_p.tile([KC, CHUNKS * QG], F8, tag="ap8")
                    pt = ps_pt.tile([KC, QG], F32, tag="pt")
                    sm = ps_sm.tile([KC, QG], F32, tag="sm")
                    # finer exp slices for the very last group shorten the tail
                    n_exp = 8 if nxt is None and g == gs[-1] else 2
                    per = PAIRS // n_exp
                    for e in range(n_exp):
                        lo, hi = e * per * 2 * QG, (e + 1) * per * 2 * QG
                        exp_i = nc.scalar.activation(
                            ap8[:, lo:hi], sp[:, lo:hi], AF.Exp, scale=-1.0)
                        if last_sqrt[0] is not None:
                            add_dep_helper(exp_i.ins, last_sqrt[0],
                                           reason="ACT table phase order")
                        last_exp[0] = exp_i.ins
                        for p in range(e * per, (e + 1) * per):
                            c0 = 2 * p
                            at_r = _r2(ap8[:, c0 * QG:(c0 + 2) * QG])
                            first = (p == 0)
                            last = (p == PAIRS - 1)
                            nc.tensor.matmul(
                                pt[0:FEAT, :],
                                _r2(bst[b]["vhi"][:, c0 * FEAT:(c0 + 2) * FEAT]),
                                at_r, start=first, stop=last, perf_mode=DR)
                            nc.tensor.matmul(
                                sm, _r2(ones8), at_r,
                                start=first, stop=last, perf_mode=DR)
                    # every row of sm holds the key-sums for its query
                    rsm_t = gout_p.tile([KC, QG], F32, tag="rsm")
                    nc.vector.reciprocal(rsm_t, sm)
                    x1_t = gout_p.tile([FEAT, QG], F32, tag="x1")
                    nc.vector.tensor_tensor(out=x1_t, in0=pt[0:FEAT, :],
                                            in1=rsm_t, op=OP.mult)
                    xt_t = gout_p.tile([FEAT, QG], F16, tag="xt")
                    # bv is folded into bo on the host (bo' = Wo@bv + bo), so
                    # xt is a plain add -- runs on the otherwise idle Pool
                    nc.gpsimd.tensor_tensor(
                        out=xt_t, in0=x1_t,
                        in1=pre_dma[(b, g)][0].bitcast(F32), op=OP.add)
                    po = ps_sm.tile([KC, QG], F32, tag="sm")
                    nc.tensor.matmul(po[0:FEAT, :], wo_t, xt_t,
                                     start=True, stop=True)
                    o_t = gout_p.tile([FEAT, QG], F32, tag="o")
                    nc.vector.tensor_scalar_add(o_t, po[0:FEAT, :], bo_t)
                    nc.sync.dma_start(out=outT.ap()[b, :, q0:q0 + QG], in_=o_t)

                    # -- pre-work for the next unit, spread through phase B --
                    if nxt is not None:
                        nb, ngs = nxt
                        if gi == 0:
                            # DMA starts only (SP queue; lands during B)
                            if nb != b:
                                batch_dmas(nb)
                            group_dmas(nb, ngs[0])
                            if nb != b:
                                batch_dmas2(nb)
                        elif gi == 1:
                            # PE/DVE pre-work after the first group's B block
                            do_qt(nb, ngs[0])
                            for p in range(NPRE):
                                pair_ds(nb, ngs[0], p)
                        if nb != b and gi == len(gs) - 1:
                            # K projection last: its kx/sf16 DMAs need time
                            batch_kproj(nb)

    nc.compile()
    _NC_CACHE["nc"] = nc
    return nc


def _prep_inputs(sparse_xyz, sparse_feat, dense_xyz, dense_feat,
                 Wq, bq, Wk, bk, Wv, bv, Wo, bo):
    """Host-side layout prep: transposes, weight folding, xyz augmentation."""
    f32 = np.float32
    Wq = Wq.astype(f32) * f32(SCALE)
    bq_s = bq.astype(f32) * f32(SCALE)

    dfT = np.ascontiguousarray(dense_feat.transpose(0, 2, 1), dtype=f32)
    dxT = np.ascontiguousarray(dense_xyz.transpose(0, 2, 1), dtype=f32)
    sfT = np.ascontiguousarray(sparse_feat.transpose(0, 2, 1), dtype=f32)
    sxT = np.ascontiguousarray(sparse_xyz.transpose(0, 2, 1), dtype=f32)

    # ds = sum_d kaug[d] * qaug[d] = 0.25 * dist^2, computed as an fp16
    # matmul.  Naive [qn, 1, -2q] x [1, kn, k] augmentation cancels
    # catastrophically once inputs are rounded (negative ds -> sqrt NaN), so
    # every value is split hi/lo into two fp16 parts; fp16 x fp16 products
    # are exact in the fp32 PSUM accumulator, leaving ~3e-6 total error.
    f16, f64 = np.float16, np.float64

    def hilo(x):
        hi = x.astype(f16)
        lo = (x - hi.astype(f64)).astype(f16)
        return hi, lo

    qn = np.sum(dense_xyz.astype(f64) ** 2, axis=-1)   # [B, N2]
    kn = np.sum(sparse_xyz.astype(f64) ** 2, axis=-1)  # [B, N1]
    qnh, qnl = hilo(qn)
    knh, knl = hilo(kn)
    qch, qcl = hilo(dxT.astype(f64))                   # [B, 3, N2] each
    kch, kcl = hilo(sxT.astype(f64))
    one1 = np.ones((B, 1, N1), f16)
    quart2 = np.full((B, 1, N2), 0.25, f16)
    qaug = np.concatenate(
        [0.25 * qnh[:, None, :].astype(f16), 0.25 * qnl[:, None, :].astype(f16),
         quart2, quart2,
         -0.5 * qch, -0.5 * qch, -0.5 * qcl, -0.5 * qcl], axis=1).astype(f16)
    kaug = np.concatenate(
        [one1, one1, knh[:, None, :], knl[:, None, :],
         kch, kcl, kch, kcl], axis=1).astype(f16)

    common = {
        "sfT16": sfT.astype(np.float16),
        # negated xyz contribution of K plus bias (kernel accumulates -st)
        "kxT": np.ascontiguousarray(
            (-(sparse_xyz.astype(f64) @ Wk[:, :3].T.astype(f64)
               + bk.astype(f64)[None, None, :])).transpose(0, 2, 1)
        ).astype(np.float16),
        "kaug": kaug,
        "WqfT": np.ascontiguousarray(Wq[:, 3:].T, f32),
        "WqxT": np.ascontiguousarray(Wq[:, :3].T, f32),
        "WkfT": np.ascontiguousarray((-Wk[:, 3:]).T.astype(np.float16)),
        "WvT": np.ascontiguousarray(Wv.T.astype(np.float16)),
        "WoT": np.ascontiguousarray(Wo.T.astype(np.float16)),
        "bq": bq_s.reshape(FEAT, 1),
        "bv": bv.astype(f32).reshape(FEAT, 1),
        # bo' = Wo @ bv + bo (bv dropped from the xt add on device)
        "bo": (Wo.astype(f64) @ bv.astype(f64)
               + bo.astype(f64)).astype(f32).reshape(FEAT, 1),
    }
    in_maps = []
    for c in range(NCORES):
        sl = slice(c * QPC, (c + 1) * QPC)
        m = dict(common)
        m["dfT"] = np.ascontiguousarray(dfT[:, :, sl])
        m["dxT"] = np.ascontiguousarray(dxT[:, :, sl])
        m["qaug"] = np.ascontiguousarray(qaug[:, :, sl])
        in_maps.append(m)
    return in_maps


def run_sharded(in_maps, trace=False):
    nc = _build()
    kwargs = {}
    if trace:
        kwargs = {"trace": True}
    return bass_utils.run_bass_kernel_spmd(
        nc, in_maps, core_ids=list(range(NCORES)), **kwargs)


def kernel(sparse_xyz, sparse_feat, dense_xyz, dense_feat,
           Wq, bq, Wk, bk, Wv, bv, Wo, bo):
    args = [np.asarray(a) for a in (sparse_xyz, sparse_feat, dense_xyz,
                                    dense_feat, Wq, bq, Wk, bk, Wv, bv,
                                    Wo, bo)]
    in_maps = _prep_inputs(*args)
    res = run_sharded(in_maps, trace=bool(os.environ.get("BASS_KERNEL_TRACE")))
    out = np.empty((B, N2, FEAT), dtype=np.float32)
    for c in range(NCORES):
        out[:, c * QPC:(c + 1) * QPC, :] = \
            res.results[c]["outT"].transpose(0, 2, 1)
    if os.environ.get("BASS_KERNEL_TRACE"):
        print("HW exec time:", res.exec_time_ns, "ns")
    return out


# revision 25
# speedup vs baseline: 1.0009x; 1.0009x over previous
"""Trainium2 Bass kernel for nn_AttentionPropagation (sparse attention propagation).

Reference computation:
  Q = cat(dense_xyz, dense_feat) @ Wq.T + bq            [B, N2, F]
  K = cat(sparse_xyz, sparse_feat) @ Wk.T + bk          [B, N1, F]
  V = sparse_feat @ Wv.T + bv                           [B, N1, F]
  attn = softmax(Q K^T / sqrt(F) - 0.5 * dist(dense_xyz, sparse_xyz))
  out = (attn @ V + dense_feat) @ Wo.T + bo             [B, N2, F]

Shapes: B=2, N1=4096 (sparse/keys), N2=32768 (dense/queries), F=128.
Sharding: queries (N2) split across 8 cores; sparse K/V + weights replicated.

Per-core kernel, transposed layout (keys on partitions, queries on free dim).
Per pair of key chunks (2x128 keys x 512 queries, one [128,1024] PSUM tile):
  ds = 0.25*dist^2            (PE, K=16 fp16 hi/lo-split aug matmul; exact)
  tile = sqrt(ds + eps)       (ACT, in place on PSUM)
  tile += -st                 (PE accumulates -K@Q^T; K negated on host)
  spre = copy(tile) -> f16    (DVE; spre = 0.5*dist - st)
  attn = exp(-spre) -> fp8e4  (ACT, scale=-1)
  P^T += V8 @ attn            (PE, fp8 DoubleRow over chunk pairs; V in
                               single-level fp8e4 -- softmax renormalization
                               cancels most of the quantization)
  sums += ones @ attn         (PE, fp8 DoubleRow, all-ones stationary ->
                               every psum row holds the sums; no broadcast)
  out^T = Wo @ ((P^T * 1/sums) + dense_feat^T) + bo'   (bo' = Wo@bv + bo,
                               folded on host; the xt add runs on Pool)
The old standalone DVE subtract pass and the Pool partition_broadcast are
gone; sqrt and exp still alternate in table-set phases over units of 2-3
query groups to amortize the ~1.3us ACT table loads.  Phase A is
software-pipelined (ds of pair i+1 ahead of st of pair i) and each unit's
phase B pre-issues the next unit's DMAs, Q projection and first ds pairs.
"""

import os
import numpy as np

os.environ.setdefault("JAX_COMPILATION_CACHE_DIR", "/tmp/jax_bass_cache")
os.environ.setdefault("JAX_PERSISTENT_CACHE_MIN_ENTRY_SIZE_BYTES", "0")
os.environ.setdefault("JAX_PERSISTENT_CACHE_MIN_COMPILE_TIME_SECS", "1")

import concourse.bacc as bacc
import concourse.tile as tile
import concourse.mybir as mybir
from concourse import bass_utils
from concourse.tile import add_dep_helper

F32 = mybir.dt.float32
F32R = mybir.dt.float32r
F16 = mybir.dt.float16
F8 = mybir.dt.float8e4
AF = mybir.ActivationFunctionType
OP = mybir.AluOpType
DR = mybir.MatmulPerfMode.DoubleRow

B = 2
N1 = 4096          # sparse points (keys)
N2 = 32768         # dense points (queries)
FEAT = 128
SCALE = FEAT ** -0.5
NCORES = 8
QPC = N2 // NCORES  # queries per core per batch (4096)
QG = 512            # query group (matmul moving free dim)
GROUPS = QPC // QG  # 8 groups per batch
KC = 128            # key chunk (PSUM partition dim)
CHUNKS = N1 // KC   # 32
PAIRS = CHUNKS // 2  # 16 chunk pairs (one [128,1024] psum tile each)
UNITS = [(0, 1), (2, 3, 4), (5, 6, 7)]  # groups per sqrt/exp table unit
NAUG = 16           # hi/lo-split dist^2 augmentation rows
SQRT_EPS = 1e-5     # covers residual fp16-split ds error (~3e-6) and the
                    # reference's 1e-12 clamp; adds <2e-3 logit error

_NC_CACHE = {}


def _r2(ap):
    # [p, (two f)] -> [p, two, f] for DoubleRow matmuls
    return ap.rearrange("p (two f) -> p two f", two=2)


def _build():
    if "nc" in _NC_CACHE:
        return _NC_CACHE["nc"]
    nc = bacc.Bacc("TRN2", target_bir_lowering=False, debug=False)

    # ---- DRAM I/O (per-core shard) ----
    dfT = nc.dram_tensor("dfT", [B, FEAT, QPC], F32R, kind="ExternalInput")
    dxT = nc.dram_tensor("dxT", [B, 3, QPC], F32R, kind="ExternalInput")
    qaug = nc.dram_tensor("qaug", [B, NAUG, QPC], F16, kind="ExternalInput")
    sfT16 = nc.dram_tensor("sfT16", [B, FEAT, N1], F16, kind="ExternalInput")
    kxT = nc.dram_tensor("kxT", [B, FEAT, N1], F16, kind="ExternalInput")  # -(xyz@WkxT+bk)
    kaug = nc.dram_tensor("kaug", [B, NAUG, N1], F16, kind="ExternalInput")
    WqfT = nc.dram_tensor("WqfT", [FEAT, FEAT], F32R, kind="ExternalInput")
    WqxT = nc.dram_tensor("WqxT", [3, FEAT], F32R, kind="ExternalInput")
    WkfT = nc.dram_tensor("WkfT", [FEAT, FEAT], F16, kind="ExternalInput")  # negated
    WvT = nc.dram_tensor("WvT", [FEAT, FEAT], F16, kind="ExternalInput")
    WoT = nc.dram_tensor("WoT", [FEAT, FEAT], F16, kind="ExternalInput")
    bq = nc.dram_tensor("bq", [FEAT, 1], F32, kind="ExternalInput")
    bv = nc.dram_tensor("bv", [FEAT, 1], F32, kind="ExternalInput")
    bo = nc.dram_tensor("bo", [FEAT, 1], F32, kind="ExternalInput")
    outT = nc.dram_tensor("outT", [B, FEAT, QPC], F32, kind="ExternalOutput")

    with tile.TileContext(nc) as tc:
        with tc.tile_pool(name="const", bufs=1) as const_p, \
             tc.tile_pool(name="batch", bufs=1) as batch_p, \
             tc.tile_pool(name="slab", bufs=3) as slab_p, \
             tc.tile_pool(name="attn", bufs=2) as attn_p, \
             tc.tile_pool(name="stage", bufs=1) as stage_p, \
             tc.tile_pool(name="gsmall", bufs=4) as gsm_p, \
             tc.tile_pool(name="gout", bufs=2) as gout_p, \
             tc.tile_pool(name="ps_pair", bufs=3, space="PSUM") as ps_pair, \
             tc.tile_pool(name="ps_pt", bufs=1, space="PSUM") as ps_pt, \
             tc.tile_pool(name="ps_sm", bufs=1, space="PSUM") as ps_sm:

            # ---- constants ----
            wqf_t = const_p.tile([FEAT, FEAT], F32R)
            wqx_t = const_p.tile([3, FEAT], F32R)
            wkf_t = const_p.tile([FEAT, FEAT], F16)
            wv_t = const_p.tile([FEAT, FEAT], F16)
            wo_t = const_p.tile([FEAT, FEAT], F16)
            bq_t = const_p.tile([FEAT, 1], F32)
            bv_t = const_p.tile([FEAT, 1], F32)
            bo_t = const_p.tile([FEAT, 1], F32)
            ones8 = const_p.tile([KC, 2 * KC], F8)
            eps_t = const_p.tile([KC, 1], F32)
            dummy_t = const_p.tile([1, 1], F32)
            nc.vector.memset(eps_t, SQRT_EPS)
            # preload the sqrt table at t=0, overlapping the input DMAs
            nc.scalar.activation(dummy_t, eps_t[0:1, 0:1], AF.Sqrt,
                                 bias=eps_t[0:1, 0:1])
            for t, d in ((wqf_t, WqfT), (wqx_t, WqxT), (wkf_t, WkfT),
                         (wv_t, WvT), (wo_t, WoT),
                         (bq_t, bq), (bv_t, bv), (bo_t, bo)):
                # constants ride the SWDGE queue to keep the HWDGE queue free
                # for the first group's inputs at startup
                nc.gpsimd.dma_start(out=t, in_=d.ap())
            nc.vector.memset(ones8, 1.0)

            # ACT runs exactly two table-based funcs (Sqrt, Exp) in different
            # table sets; a table switch costs ~1.3us.  Pin every sqrt of
            # unit u+1 behind the last exp of unit u so the scheduler cannot
            # interleave the phases.
            last_exp = [None]
            last_sqrt = [None]

            bst = {}      # batch -> dict of per-batch slabs
            pre_dma = {}  # (b, g) -> (df_t, dx_t, qa_t)
            pre_qt = {}   # (b, g) -> qt tile
            pre_ds = {}   # (b, g, p) -> pair psum tile (ds done, sqrt pending)

            def batch_dmas(b):
                st = {}
                st["sf16"] = stage_p.tile([FEAT, N1], F16, tag="sf16", name="sf16")
                st["kx"] = stage_p.tile([FEAT, N1], F16, tag="kx", name="kx")
                st["ka"] = batch_p.tile([NAUG, N1], F16, tag="ka", name="ka")
                nc.sync.dma_start(out=st["ka"], in_=kaug.ap()[b])
                bst[b] = st
                return st

            def batch_dmas2(b):
                st = bst[b]
                nc.sync.dma_start(out=st["kx"], in_=kxT.ap()[b])
                # chunked so the j=0 K^T projection starts on the first slice
                for j in range(N1 // QG):
                    nc.sync.dma_start(
                        out=st["sf16"][:, j * QG:(j + 1) * QG],
                        in_=sfT16.ap()[b, :, j * QG:(j + 1) * QG])

            def kproj_alloc(b):
                st = bst[b]
                st["kt"] = batch_p.tile([FEAT, N1], F16, tag="kt", name="kt")  # -K^T
                st["vhi"] = batch_p.tile([KC, CHUNKS * FEAT], F8, tag="vhi", name="vhi")
                st["kproj_j"] = 0

            def kproj_step(b, upto_j):
                st = bst[b]
                while st["kproj_j"] <= min(upto_j, N1 // QG - 1):
                    j = st["kproj_j"]
                    ps = ps_pair.tile([KC, 2 * QG], F32, tag="pair")
                    nc.tensor.matmul(ps[:, 0:QG], wkf_t,
                                     st["sf16"][:, j * QG:(j + 1) * QG],
                                     start=True, stop=True)
                    # xyz part of -K plus bias, folded on the host into kxT
                    nc.vector.tensor_tensor(
                        out=st["kt"][:, j * QG:(j + 1) * QG], in0=ps[:, 0:QG],
                        in1=st["kx"][:, j * QG:(j + 1) * QG], op=OP.add)
                    st["kproj_j"] = j + 1

            def batch_kproj(b):
                kproj_alloc(b)
                kproj_step(b, N1 // QG - 1)

            def project_v(b):
                # V is only read in phase B; emitting it after the first
                # unit's phase A keeps PE on the critical path early on.
                # Single-level fp8 V: softmax renormalization cancels most
                # of the quantization (golden model: +1e-4 rel err).
                st = bst[b]
                for c in range(CHUNKS):
                    ps = ps_pt.tile([KC, QG], F32, tag="pt")
                    nc.tensor.matmul(ps[:, 0:FEAT],
                                     st["sf16"][:, c * KC:(c + 1) * KC], wv_t,
                                     start=True, stop=True)
                    nc.vector.tensor_copy(
                        st["vhi"][:, c * FEAT:(c + 1) * FEAT], ps[:, 0:FEAT])

            def group_dmas(b, g):
                q0 = g * QG
                df_t = gsm_p.tile([FEAT, QG], F32R, tag="df", bufs=4)
                dx_t = gsm_p.tile([3, QG], F32R, tag="dx")
                qa_t = gsm_p.tile([NAUG, QG], F16, tag="qa")
                # qa first: it gates the ds matmuls -> sqrt pipeline
                nc.sync.dma_start(out=qa_t, in_=qaug.ap()[b, :, q0:q0 + QG])
                nc.sync.dma_start(out=dx_t, in_=dxT.ap()[b, :, q0:q0 + QG])
                nc.sync.dma_start(out=df_t, in_=dfT.ap()[b, :, q0:q0 + QG])
                r = (df_t, dx_t, qa_t)
                pre_dma[(b, g)] = r
                return r

            def do_qt(b, g):
                df_t, dx_t, qa_t = pre_dma[(b, g)]
                ps_q = ps_pair.tile([KC, 2 * QG], F32, tag="pair")
                nc.tensor.matmul(ps_q[:, 0:QG], wqf_t, df_t,
                                 start=True, stop=False)
                nc.tensor.matmul(ps_q[:, 0:QG], wqx_t, dx_t,
                                 start=False, stop=True)
                qt_t = gsm_p.tile([FEAT, QG], F16, tag="qt")
                nc.vector.tensor_scalar_add(qt_t, ps_q[:, 0:QG], bq_t)
                pre_qt[(b, g)] = qt_t
                return qt_t

            def pair_ds(b, g, p):
                """ds matmuls into a fresh pair tile (PE only)."""
                qa_t = pre_dma[(b, g)][2]
                ka_t = bst[b]["ka"]
                pt_ps = ps_pair.tile([KC, 2 * QG], F32, tag="pair")
                for ci, c in ((0, 2 * p), (1, 2 * p + 1)):
                    nc.tensor.matmul(pt_ps[:, ci * QG:(ci + 1) * QG],
                                     ka_t[:, c * KC:(c + 1) * KC],
                                     qa_t, start=True, stop=True)
                pre_ds[(b, g, p)] = pt_ps
                return pt_ps

            def pair_sqrt(pt_ps):
                sq_i = nc.scalar.activation(pt_ps, pt_ps, AF.Sqrt,
                                            bias=eps_t[:, 0:1])
                if last_exp[0] is not None:
                    add_dep_helper(sq_i.ins, last_exp[0],
                                   reason="ACT table phase order")
                last_sqrt[0] = sq_i.ins

            def pair_back(b, g, p, pt_ps, sp):
                """-st accumulate onto the sqrt'd tile + f16 copy out."""
                qt_t = pre_qt[(b, g)]
                kt_t = bst[b]["kt"]
                c0 = 2 * p
                for ci, c in ((0, c0), (1, c0 + 1)):
                    nc.tensor.matmul(pt_ps[:, ci * QG:(ci + 1) * QG],
                                     kt_t[:, c * KC:(c + 1) * KC], qt_t,
                                     start=False, stop=True,
                                     skip_group_check=True)
                nc.vector.tensor_copy(sp[:, c0 * QG:(c0 + 2) * QG], pt_ps)

            units_all = [(b, gs) for b in range(B) for gs in UNITS]
            NPRE = 2  # pair tiles pre-ds'ed for the next unit during phase B

            # ---- startup: first group's DMAs + Q proj, then batch-0 setup ----
            batch_dmas(0)
            group_dmas(0, 0)
            do_qt(0, 0)
            batch_dmas2(0)
            batch_kproj(0)

            for ui, (b, gs) in enumerate(units_all):
                first_of_batch = (gs is UNITS[0])
                nxt = units_all[ui + 1] if ui + 1 < len(units_all) else None

                # ---------- phase A: ds -> sqrt -> -st -> f16 copy ----------
                # Software-pipelined: the ds+sqrt of pair i+1 is emitted
                # before the st matmuls of pair i, so the in-order PE queue
                # never waits on the ACT sqrt.
                spre = {}
                work = []  # (b, g, p, pt_ps, sp) awaiting back-half
                for g in gs:
                    if (b, g) not in pre_dma:
                        group_dmas(b, g)
                    if (b, g) not in pre_qt:
                        do_qt(b, g)
                    sp = slab_p.tile([KC, CHUNKS * QG], F16, tag="spre")
                    spre[g] = sp
                    for p in range(PAIRS):
                        pt_ps = pre_ds.pop((b, g, p), None)
                        if pt_ps is None:
                            pt_ps = pair_ds(b, g, p)
                        pair_sqrt(pt_ps)
                        work.append((b, g, p, pt_ps, sp))
                        if len(work) > 1:
                            pair_back(*work.pop(0))
                while work:
                    pair_back(*work.pop(0))

                if first_of_batch:
                    project_v(b)

                # ---------- phase B: exp -> attn@V (fp8 DR) -> out ----------
                for gi, g in enumerate(gs):
                    q0 = g * QG
                    sp = spre[g]
                    ap8 = attn_p.tile([KC, CHUNKS * QG], F8, tag="ap8")
                    pt = ps_pt.tile([KC, QG], F32, tag="pt")
                    sm = ps_sm.tile([KC, QG], F32, tag="sm")
                    # finer exp slices for the very last group shorten the tail
                    n_exp = 8 if nxt is None and g == gs[-1] else 2
                    per = PAIRS // n_exp
                    for e in range(n_exp):
                        lo, hi = e * per * 2 * QG, (e + 1) * per * 2 * QG
                        exp_i = nc.scalar.activation(
                            ap8[:, lo:hi], sp[:, lo:hi], AF.Exp, scale=-1.0)
                        if last_sqrt[0] is not None:
                            add_dep_helper(exp_i.ins, last_sqrt[0],
                                           reason="ACT table phase order")
                        last_exp[0] = exp_i.ins
                        for p in range(e * per, (e + 1) * per):
                            c0 = 2 * p
                            at_r = _r2(ap8[:, c0 * QG:(c0 + 2) * QG])
                            first = (p == 0)
                            last = (p == PAIRS - 1)
                            nc.tensor.matmul(
                                pt[0:FEAT, :],
                                _r2(bst[b]["vhi"][:, c0 * FEAT:(c0 + 2) * FEAT]),
                                at_r, start=first, stop=last, perf_mode=DR)
                            nc.tensor.matmul(
                                sm, _r2(ones8), at_r,
                                start=first, stop=last, perf_mode=DR)
                    # copy P^T out of PSUM first so the single pt bank is
                    # free for the next group's DoubleRow accumulation
                    xc_t = gout_p.tile([FEAT, QG], F16, tag="xc")
                    nc.vector.tensor_copy(xc_t, pt[0:FEAT, :])
                    # every row of sm holds the key-sums for its query
                    rsm_t = gout_p.tile([KC, QG], F32, tag="rsm")
                    nc.vector.reciprocal(rsm_t, sm)
                    x1_t = gout_p.tile([FEAT, QG], F32, tag="x1")
                    nc.vector.tensor_tensor(out=x1_t, in0=xc_t,
                                            in1=rsm_t, op=OP.mult)
                    xt_t = gout_p.tile([FEAT, QG], F16, tag="xt")
                    # bv is folded into bo on the host (bo' = Wo@bv + bo), so
                    # xt is a plain add -- runs on the otherwise idle Pool
                    nc.gpsimd.tensor_tensor(
                        out=xt_t, in0=x1_t,
                        in1=pre_dma[(b, g)][0].bitcast(F32), op=OP.add)
                    po = ps_sm.tile([KC, QG], F32, tag="sm")
                    nc.tensor.matmul(po[0:FEAT, :], wo_t, xt_t,
                                     start=True, stop=True)
                    o_t = gout_p.tile([FEAT, QG], F32, tag="o")
                    nc.vector.tensor_scalar_add(o_t, po[0:FEAT, :], bo_t)
                    nc.sync.dma_start(out=outT.ap()[b, :, q0:q0 + QG], in_=o_t)

                    # -- pre-work for the next unit, spread through phase B --
                    if nxt is not None:
                        nb, ngs = nxt
                        if gi == 0:
                            # DMA starts only (SP queue; lands during B)
                            if nb != b:
                                batch_dmas(nb)
                            group_dmas(nb, ngs[0])
                            if nb != b:
                                batch_dmas2(nb)
                        elif gi == 1:
                            # PE/DVE pre-work after the first group's B block
                            do_qt(nb, ngs[0])
                            for p in range(NPRE):
                                pair_ds(nb, ngs[0], p)
                        if nb != b and gi == len(gs) - 1:
                            # K projection last: its kx/sf16 DMAs need time
                            batch_kproj(nb)

    nc.compile()
    _NC_CACHE["nc"] = nc
    return nc


def _prep_inputs(sparse_xyz, sparse_feat, dense_xyz, dense_feat,
                 Wq, bq, Wk, bk, Wv, bv, Wo, bo):
    """Host-side layout prep: transposes, weight folding, xyz augmentation."""
    f32 = np.float32
    Wq = Wq.astype(f32) * f32(SCALE)
    bq_s = bq.astype(f32) * f32(SCALE)

    dfT = np.ascontiguousarray(dense_feat.transpose(0, 2, 1), dtype=f32)
    dxT = np.ascontiguousarray(dense_xyz.transpose(0, 2, 1), dtype=f32)
    sfT = np.ascontiguousarray(sparse_feat.transpose(0, 2, 1), dtype=f32)
    sxT = np.ascontiguousarray(sparse_xyz.transpose(0, 2, 1), dtype=f32)

    # ds = sum_d kaug[d] * qaug[d] = 0.25 * dist^2, computed as an fp16
    # matmul.  Naive [qn, 1, -2q] x [1, kn, k] augmentation cancels
    # catastrophically once inputs are rounded (negative ds -> sqrt NaN), so
    # every value is split hi/lo into two fp16 parts; fp16 x fp16 products
    # are exact in the fp32 PSUM accumulator, leaving ~3e-6 total error.
    f16, f64 = np.float16, np.float64

    def hilo(x):
        hi = x.astype(f16)
        lo = (x - hi.astype(f64)).astype(f16)
        return hi, lo

    qn = np.sum(dense_xyz.astype(f64) ** 2, axis=-1)   # [B, N2]
    kn = np.sum(sparse_xyz.astype(f64) ** 2, axis=-1)  # [B, N1]
    qnh, qnl = hilo(qn)
    knh, knl = hilo(kn)
    qch, qcl = hilo(dxT.astype(f64))                   # [B, 3, N2] each
    kch, kcl = hilo(sxT.astype(f64))
    one1 = np.ones((B, 1, N1), f16)
    quart2 = np.full((B, 1, N2), 0.25, f16)
    qaug = np.concatenate(
        [0.25 * qnh[:, None, :].astype(f16), 0.25 * qnl[:, None, :].astype(f16),
         quart2, quart2,
         -0.5 * qch, -0.5 * qch, -0.5 * qcl, -0.5 * qcl], axis=1).astype(f16)
    kaug = np.concatenate(
        [one1, one1, knh[:, None, :], knl[:, None, :],
         kch, kcl, kch, kcl], axis=1).astype(f16)

    common = {
        "sfT16": sfT.astype(np.float16),
        # negated xyz contribution of K plus bias (kernel accumulates -st)
        "kxT": np.ascontiguousarray(
            (-(sparse_xyz.astype(f64) @ Wk[:, :3].T.astype(f64)
               + bk.astype(f64)[None, None, :])).transpose(0, 2, 1)
        ).astype(np.float16),
        "kaug": kaug,
        "WqfT": np.ascontiguousarray(Wq[:, 3:].T, f32),
        "WqxT": np.ascontiguousarray(Wq[:, :3].T, f32),
        "WkfT": np.ascontiguousarray((-Wk[:, 3:]).T.astype(np.float16)),
        "WvT": np.ascontiguousarray(Wv.T.astype(np.float16)),
        "WoT": np.ascontiguousarray(Wo.T.astype(np.float16)),
        "bq": bq_s.reshape(FEAT, 1),
        "bv": bv.astype(f32).reshape(FEAT, 1),
        # bo' = Wo @ bv + bo (bv dropped from the xt add on device)
        "bo": (Wo.astype(f64) @ bv.astype(f64)
               + bo.astype(f64)).astype(f32).reshape(FEAT, 1),
    }
    in_maps = []
    for c in range(NCORES):
        sl = slice(c * QPC, (c + 1) * QPC)
        m = dict(common)
        m["dfT"] = np.ascontiguousarray(dfT[:, :, sl])
        m["dxT"] = np.ascontiguousarray(dxT[:, :, sl])
        m["qaug"] = np.ascontiguousarray(qaug[:, :, sl])
        in_maps.append(m)
    return in_maps


def run_sharded(in_maps, trace=False):
    nc = _build()
    kwargs = {}
    if trace:
        kwargs = {"trace": True}
    return bass_utils.run_bass_kernel_spmd(
        nc, in_maps, core_ids=list(range(NCORES)), **kwargs)


def kernel(sparse_xyz, sparse_feat, dense_xyz, dense_feat,
           Wq, bq, Wk, bk, Wv, bv, Wo, bo):
    args = [np.asarray(a) for a in (sparse_xyz, sparse_feat, dense_xyz,
                                    dense_feat, Wq, bq, Wk, bk, Wv, bv,
                                    Wo, bo)]
    in_maps = _prep_inputs(*args)
    res = run_sharded(in_maps, trace=bool(os.environ.get("BASS_KERNEL_TRACE")))
    out = np.empty((B, N2, FEAT), dtype=np.float32)
    for c in range(NCORES):
        out[:, c * QPC:(c + 1) * QPC, :] = \
            res.results[c]["outT"].transpose(0, 2, 1)
    if os.environ.get("BASS_KERNEL_TRACE"):
        print("HW exec time:", res.exec_time_ns, "ns")
    return out


# revision 26
# speedup vs baseline: 1.0016x; 1.0007x over previous
"""Trainium2 Bass kernel for nn_AttentionPropagation (sparse attention propagation).

Reference computation:
  Q = cat(dense_xyz, dense_feat) @ Wq.T + bq            [B, N2, F]
  K = cat(sparse_xyz, sparse_feat) @ Wk.T + bk          [B, N1, F]
  V = sparse_feat @ Wv.T + bv                           [B, N1, F]
  attn = softmax(Q K^T / sqrt(F) - 0.5 * dist(dense_xyz, sparse_xyz))
  out = (attn @ V + dense_feat) @ Wo.T + bo             [B, N2, F]

Shapes: B=2, N1=4096 (sparse/keys), N2=32768 (dense/queries), F=128.
Sharding: queries (N2) split across 8 cores; sparse K/V + weights replicated.

Per-core kernel, transposed layout (keys on partitions, queries on free dim).
Per pair of key chunks (2x128 keys x 512 queries, one [128,1024] PSUM tile):
  ds = 0.25*dist^2            (PE, K=16 fp16 hi/lo-split aug matmul; exact)
  tile = sqrt(ds + eps)       (ACT, in place on PSUM)
  tile += -st                 (PE accumulates -K@Q^T; K negated on host)
  spre = copy(tile) -> f16    (DVE; spre = 0.5*dist - st)
  attn = exp(-spre) -> fp8e4  (ACT, scale=-1)
  P^T += V8 @ attn            (PE, fp8 DoubleRow over chunk pairs; V in
                               single-level fp8e4 -- softmax renormalization
                               cancels most of the quantization)
  sums += ones @ attn         (PE, fp8 DoubleRow, all-ones stationary ->
                               every psum row holds the sums; no broadcast)
  out^T = Wo @ ((P^T * 1/sums) + dense_feat^T) + bo'   (bo' = Wo@bv + bo,
                               folded on host; the xt add runs on Pool)
The old standalone DVE subtract pass and the Pool partition_broadcast are
gone; sqrt and exp still alternate in table-set phases over units of 2-3
query groups to amortize the ~1.3us ACT table loads.  Phase A is
software-pipelined (ds of pair i+1 ahead of st of pair i) and each unit's
phase B pre-issues the next unit's DMAs, Q projection and first ds pairs.
"""

import os
import numpy as np

os.environ.setdefault("JAX_COMPILATION_CACHE_DIR", "/tmp/jax_bass_cache")
os.environ.setdefault("JAX_PERSISTENT_CACHE_MIN_ENTRY_SIZE_BYTES", "0")
os.environ.setdefault("JAX_PERSISTENT_CACHE_MIN_COMPILE_TIME_SECS", "1")

import concourse.bacc as bacc
import concourse.tile as tile
import concourse.mybir as mybir
from concourse import bass_utils
from concourse.tile import add_dep_helper

F32 = mybir.dt.float32
F32R = mybir.dt.float32r
F16 = mybir.dt.float16
F8 = mybir.dt.float8e4
AF = mybir.ActivationFunctionType
OP = mybir.AluOpType
DR = mybir.MatmulPerfMode.DoubleRow

B = 2
N1 = 4096          # sparse points (keys)
N2 = 32768         # dense points (queries)
FEAT = 128
SCALE = FEAT ** -0.5
NCORES = 8
QPC = N2 // NCORES  # queries per core per batch (4096)
QG = 512            # query group (matmul moving free dim)
GROUPS = QPC // QG  # 8 groups per batch
KC = 128            # key chunk (PSUM partition dim)
CHUNKS = N1 // KC   # 32
PAIRS = CHUNKS // 2  # 16 chunk pairs (one [128,1024] psum tile each)
UNITS = [(0, 1), (2, 3, 4), (5, 6, 7)]  # groups per sqrt/exp table unit
NAUG = 16           # hi/lo-split dist^2 augmentation rows
SQRT_EPS = 1e-5     # covers residual fp16-split ds error (~3e-6) and the
                    # reference's 1e-12 clamp; adds <2e-3 logit error

_NC_CACHE = {}


def _r2(ap):
    # [p, (two f)] -> [p, two, f] for DoubleRow matmuls
    return ap.rearrange("p (two f) -> p two f", two=2)


def _build():
    if "nc" in _NC_CACHE:
        return _NC_CACHE["nc"]
    nc = bacc.Bacc("TRN2", target_bir_lowering=False, debug=False)

    # ---- DRAM I/O (per-core shard) ----
    dfT = nc.dram_tensor("dfT", [B, FEAT, QPC], F32R, kind="ExternalInput")
    dxT = nc.dram_tensor("dxT", [B, 3, QPC], F32R, kind="ExternalInput")
    qaug = nc.dram_tensor("qaug", [B, NAUG, QPC], F16, kind="ExternalInput")
    sfT16 = nc.dram_tensor("sfT16", [B, FEAT, N1], F16, kind="ExternalInput")
    kxT = nc.dram_tensor("kxT", [B, FEAT, N1], F16, kind="ExternalInput")  # -(xyz@WkxT+bk)
    kaug = nc.dram_tensor("kaug", [B, NAUG, N1], F16, kind="ExternalInput")
    WqfT = nc.dram_tensor("WqfT", [FEAT, FEAT], F32R, kind="ExternalInput")
    WqxT = nc.dram_tensor("WqxT", [3, FEAT], F32R, kind="ExternalInput")
    WkfT = nc.dram_tensor("WkfT", [FEAT, FEAT], F16, kind="ExternalInput")  # negated
    WvT = nc.dram_tensor("WvT", [FEAT, FEAT], F16, kind="ExternalInput")
    WoT = nc.dram_tensor("WoT", [FEAT, FEAT], F16, kind="ExternalInput")
    bq = nc.dram_tensor("bq", [FEAT, 1], F32, kind="ExternalInput")
    bv = nc.dram_tensor("bv", [FEAT, 1], F32, kind="ExternalInput")
    bo = nc.dram_tensor("bo", [FEAT, 1], F32, kind="ExternalInput")
    outT = nc.dram_tensor("outT", [B, FEAT, QPC], F32, kind="ExternalOutput")

    with tile.TileContext(nc) as tc:
        with tc.tile_pool(name="const", bufs=1) as const_p, \
             tc.tile_pool(name="batch", bufs=1) as batch_p, \
             tc.tile_pool(name="slab", bufs=3) as slab_p, \
             tc.tile_pool(name="attn", bufs=2) as attn_p, \
             tc.tile_pool(name="stage", bufs=1) as stage_p, \
             tc.tile_pool(name="gsmall", bufs=4) as gsm_p, \
             tc.tile_pool(name="gout", bufs=2) as gout_p, \
             tc.tile_pool(name="ps_pair", bufs=3, space="PSUM") as ps_pair, \
             tc.tile_pool(name="ps_pt", bufs=1, space="PSUM") as ps_pt, \
             tc.tile_pool(name="ps_sm", bufs=1, space="PSUM") as ps_sm:

            # ---- constants ----
            wqf_t = const_p.tile([FEAT, FEAT], F32R)
            wqx_t = const_p.tile([3, FEAT], F32R)
            wkf_t = const_p.tile([FEAT, FEAT], F16)
            wv_t = const_p.tile([FEAT, FEAT], F16)
            wo_t = const_p.tile([FEAT, FEAT], F16)
            bq_t = const_p.tile([FEAT, 1], F32)
            bv_t = const_p.tile([FEAT, 1], F32)
            bo_t = const_p.tile([FEAT, 1], F32)
            ones8 = const_p.tile([KC, 2 * KC], F8)
            eps_t = const_p.tile([KC, 1], F32)
            dummy_t = const_p.tile([1, 1], F32)
            nc.vector.memset(eps_t, SQRT_EPS)
            # preload the sqrt table at t=0, overlapping the input DMAs
            nc.scalar.activation(dummy_t, eps_t[0:1, 0:1], AF.Sqrt,
                                 bias=eps_t[0:1, 0:1])
            for t, d in ((wqf_t, WqfT), (wqx_t, WqxT), (wkf_t, WkfT),
                         (wv_t, WvT), (wo_t, WoT),
                         (bq_t, bq), (bv_t, bv), (bo_t, bo)):
                # constants ride the SWDGE queue to keep the HWDGE queue free
                # for the first group's inputs at startup
                nc.gpsimd.dma_start(out=t, in_=d.ap())
            nc.vector.memset(ones8, 1.0)

            # ACT runs exactly two table-based funcs (Sqrt, Exp) in different
            # table sets; a table switch costs ~1.3us.  Pin every sqrt of
            # unit u+1 behind the last exp of unit u so the scheduler cannot
            # interleave the phases.
            last_exp = [None]
            last_sqrt = [None]

            bst = {}      # batch -> dict of per-batch slabs
            pre_dma = {}  # (b, g) -> (df_t, dx_t, qa_t)
            pre_qt = {}   # (b, g) -> qt tile
            pre_ds = {}   # (b, g, p) -> pair psum tile (ds done, sqrt pending)

            def batch_dmas(b):
                st = {}
                st["sf16"] = stage_p.tile([FEAT, N1], F16, tag="sf16", name="sf16")
                st["kx"] = stage_p.tile([FEAT, N1], F16, tag="kx", name="kx")
                st["ka"] = batch_p.tile([NAUG, N1], F16, tag="ka", name="ka")
                nc.sync.dma_start(out=st["ka"], in_=kaug.ap()[b])
                bst[b] = st
                return st

            def batch_dmas2(b):
                st = bst[b]
                nc.sync.dma_start(out=st["kx"], in_=kxT.ap()[b])
                # chunked so the j=0 K^T projection starts on the first slice
                for j in range(N1 // QG):
                    nc.sync.dma_start(
                        out=st["sf16"][:, j * QG:(j + 1) * QG],
                        in_=sfT16.ap()[b, :, j * QG:(j + 1) * QG])

            def kproj_alloc(b):
                st = bst[b]
                st["kt"] = batch_p.tile([FEAT, N1], F16, tag="kt", name="kt")  # -K^T
                st["vhi"] = batch_p.tile([KC, CHUNKS * FEAT], F8, tag="vhi", name="vhi")
                st["kproj_j"] = 0

            def kproj_step(b, upto_j):
                st = bst[b]
                while st["kproj_j"] <= min(upto_j, N1 // QG - 1):
                    j = st["kproj_j"]
                    ps = ps_pair.tile([KC, 2 * QG], F32, tag="pair")
                    nc.tensor.matmul(ps[:, 0:QG], wkf_t,
                                     st["sf16"][:, j * QG:(j + 1) * QG],
                                     start=True, stop=True)
                    # xyz part of -K plus bias, folded on the host into kxT
                    nc.vector.tensor_tensor(
                        out=st["kt"][:, j * QG:(j + 1) * QG], in0=ps[:, 0:QG],
                        in1=st["kx"][:, j * QG:(j + 1) * QG], op=OP.add)
                    st["kproj_j"] = j + 1

            def batch_kproj(b):
                kproj_alloc(b)
                kproj_step(b, N1 // QG - 1)

            def project_v(b):
                # V is only read in phase B; emitting it after the first
                # unit's phase A keeps PE on the critical path early on.
                # Single-level fp8 V: softmax renormalization cancels most
                # of the quantization (golden model: +1e-4 rel err).
                st = bst[b]
                for c in range(CHUNKS):
                    ps = ps_pt.tile([KC, QG], F32, tag="pt")
                    nc.tensor.matmul(ps[:, 0:FEAT],
                                     st["sf16"][:, c * KC:(c + 1) * KC], wv_t,
                                     start=True, stop=True)
                    nc.vector.tensor_copy(
                        st["vhi"][:, c * FEAT:(c + 1) * FEAT], ps[:, 0:FEAT])

            def group_dmas(b, g):
                q0 = g * QG
                df_t = gsm_p.tile([FEAT, QG], F32R, tag="df", bufs=4)
                dx_t = gsm_p.tile([3, QG], F32R, tag="dx")
                qa_t = gsm_p.tile([NAUG, QG], F16, tag="qa")
                # qa first: it gates the ds matmuls -> sqrt pipeline
                nc.sync.dma_start(out=qa_t, in_=qaug.ap()[b, :, q0:q0 + QG])
                nc.sync.dma_start(out=dx_t, in_=dxT.ap()[b, :, q0:q0 + QG])
                nc.sync.dma_start(out=df_t, in_=dfT.ap()[b, :, q0:q0 + QG])
                r = (df_t, dx_t, qa_t)
                pre_dma[(b, g)] = r
                return r

            def do_qt(b, g):
                df_t, dx_t, qa_t = pre_dma[(b, g)]
                ps_q = ps_pair.tile([KC, 2 * QG], F32, tag="pair")
                nc.tensor.matmul(ps_q[:, 0:QG], wqf_t, df_t,
                                 start=True, stop=False)
                nc.tensor.matmul(ps_q[:, 0:QG], wqx_t, dx_t,
                                 start=False, stop=True)
                qt_t = gsm_p.tile([FEAT, QG], F16, tag="qt")
                nc.vector.tensor_scalar_add(qt_t, ps_q[:, 0:QG], bq_t)
                pre_qt[(b, g)] = qt_t
                return qt_t

            def pair_ds(b, g, p):
                """ds matmuls into a fresh pair tile (PE only)."""
                qa_t = pre_dma[(b, g)][2]
                ka_t = bst[b]["ka"]
                pt_ps = ps_pair.tile([KC, 2 * QG], F32, tag="pair")
                for ci, c in ((0, 2 * p), (1, 2 * p + 1)):
                    nc.tensor.matmul(pt_ps[:, ci * QG:(ci + 1) * QG],
                                     ka_t[:, c * KC:(c + 1) * KC],
                                     qa_t, start=True, stop=True)
                pre_ds[(b, g, p)] = pt_ps
                return pt_ps

            def pair_sqrt(pt_ps):
                sq_i = nc.scalar.activation(pt_ps, pt_ps, AF.Sqrt,
                                            bias=eps_t[:, 0:1])
                if last_exp[0] is not None:
                    add_dep_helper(sq_i.ins, last_exp[0],
                                   reason="ACT table phase order")
                last_sqrt[0] = sq_i.ins

            def pair_back(b, g, p, pt_ps, sp):
                """-st accumulate onto the sqrt'd tile + f16 copy out."""
                qt_t = pre_qt[(b, g)]
                kt_t = bst[b]["kt"]
                c0 = 2 * p
                for ci, c in ((0, c0), (1, c0 + 1)):
                    nc.tensor.matmul(pt_ps[:, ci * QG:(ci + 1) * QG],
                                     kt_t[:, c * KC:(c + 1) * KC], qt_t,
                                     start=False, stop=True,
                                     skip_group_check=True)
                nc.vector.tensor_copy(sp[:, c0 * QG:(c0 + 2) * QG], pt_ps)

            units_all = [(b, gs) for b in range(B) for gs in UNITS]
            NPRE = 2  # pair tiles pre-ds'ed for the next unit during phase B

            # ---- startup: first group's DMAs + Q proj, then batch-0 setup ----
            batch_dmas(0)
            group_dmas(0, 0)
            do_qt(0, 0)
            batch_dmas2(0)
            batch_kproj(0)

            for ui, (b, gs) in enumerate(units_all):
                first_of_batch = (gs is UNITS[0])
                nxt = units_all[ui + 1] if ui + 1 < len(units_all) else None

                # ---------- phase A: ds -> sqrt -> -st -> f16 copy ----------
                # Software-pipelined: the ds+sqrt of pair i+1 is emitted
                # before the st matmuls of pair i, so the in-order PE queue
                # never waits on the ACT sqrt.
                spre = {}
                work = []  # (b, g, p, pt_ps, sp) awaiting back-half
                for g in gs:
                    if (b, g) not in pre_dma:
                        group_dmas(b, g)
                    if (b, g) not in pre_qt:
                        do_qt(b, g)
                    sp = slab_p.tile([KC, CHUNKS * QG], F16, tag="spre")
                    spre[g] = sp
                    for p in range(PAIRS):
                        pt_ps = pre_ds.pop((b, g, p), None)
                        if pt_ps is None:
                            pt_ps = pair_ds(b, g, p)
                        pair_sqrt(pt_ps)
                        work.append((b, g, p, pt_ps, sp))
                        if len(work) > 1:
                            pair_back(*work.pop(0))
                while work:
                    pair_back(*work.pop(0))

                if first_of_batch:
                    project_v(b)

                # ---------- phase B: exp -> attn@V (fp8 DR) -> out ----------
                for gi, g in enumerate(gs):
                    q0 = g * QG
                    sp = spre[g]
                    ap8 = attn_p.tile([KC, CHUNKS * QG], F8, tag="ap8")
                    pt = ps_pt.tile([KC, QG], F32, tag="pt")
                    sm = ps_sm.tile([KC, QG], F32, tag="sm")
                    # finer exp slices for the very last group shorten the tail
                    n_exp = 8 if nxt is None and g == gs[-1] else 2
                    per = PAIRS // n_exp
                    for e in range(n_exp):
                        lo, hi = e * per * 2 * QG, (e + 1) * per * 2 * QG
                        exp_i = nc.scalar.activation(
                            ap8[:, lo:hi], sp[:, lo:hi], AF.Exp, scale=-1.0)
                        if last_sqrt[0] is not None:
                            add_dep_helper(exp_i.ins, last_sqrt[0],
                                           reason="ACT table phase order")
                        last_exp[0] = exp_i.ins
                        for p in range(e * per, (e + 1) * per):
                            c0 = 2 * p
                            at_r = _r2(ap8[:, c0 * QG:(c0 + 2) * QG])
                            first = (p == 0)
                            last = (p == PAIRS - 1)
                            nc.tensor.matmul(
                                pt[0:FEAT, :],
                                _r2(bst[b]["vhi"][:, c0 * FEAT:(c0 + 2) * FEAT]),
                                at_r, start=first, stop=last, perf_mode=DR)
                            nc.tensor.matmul(
                                sm, _r2(ones8), at_r,
                                start=first, stop=last, perf_mode=DR)
                    last_grp = nxt is None and g == gs[-1]
                    if not last_grp:
                        # copy P^T out of PSUM so the single pt bank is free
                        # for the next group's DoubleRow accumulation
                        xc_t = gout_p.tile([FEAT, QG], F16, tag="xc")
                        nc.vector.tensor_copy(xc_t, pt[0:FEAT, :])
                        pt_src = xc_t
                    else:
                        pt_src = pt[0:FEAT, :]  # skip the copy on the tail
                    # every row of sm holds the key-sums for its query
                    rsm_t = gout_p.tile([KC, QG], F32, tag="rsm")
                    nc.vector.reciprocal(rsm_t, sm)
                    x1_t = gout_p.tile([FEAT, QG], F32, tag="x1")
                    nc.vector.tensor_tensor(out=x1_t, in0=pt_src,
                                            in1=rsm_t, op=OP.mult)
                    xt_t = gout_p.tile([FEAT, QG], F16, tag="xt")
                    # bv is folded into bo on the host (bo' = Wo@bv + bo), so
                    # xt is a plain add -- runs on the otherwise idle Pool
                    nc.gpsimd.tensor_tensor(
                        out=xt_t, in0=x1_t,
                        in1=pre_dma[(b, g)][0].bitcast(F32), op=OP.add)
                    po = ps_sm.tile([KC, QG], F32, tag="sm")
                    nc.tensor.matmul(po[0:FEAT, :], wo_t, xt_t,
                                     start=True, stop=True)
                    o_t = gout_p.tile([FEAT, QG], F32, tag="o")
                    nc.vector.tensor_scalar_add(o_t, po[0:FEAT, :], bo_t)
                    nc.sync.dma_start(out=outT.ap()[b, :, q0:q0 + QG], in_=o_t)

                    # -- pre-work for the next unit, spread through phase B --
                    if nxt is not None:
                        nb, ngs = nxt
                        if gi == 0:
                            # DMA starts only (SP queue; lands during B)
                            if nb != b:
                                batch_dmas(nb)
                            group_dmas(nb, ngs[0])
                            if nb != b:
                                batch_dmas2(nb)
                        elif gi == 1:
                            # PE/DVE pre-work after the first group's B block
                            do_qt(nb, ngs[0])
                            for p in range(NPRE):
                                pair_ds(nb, ngs[0], p)
                        if nb != b and gi == len(gs) - 1:
                            # K projection last: its kx/sf16 DMAs need time
                            batch_kproj(nb)

    nc.compile()
    _NC_CACHE["nc"] = nc
    return nc


def _prep_inputs(sparse_xyz, sparse_feat, dense_xyz, dense_feat,
                 Wq, bq, Wk, bk, Wv, bv, Wo, bo):
    """Host-side layout prep: transposes, weight folding, xyz augmentation."""
    f32 = np.float32
    Wq = Wq.astype(f32) * f32(SCALE)
    bq_s = bq.astype(f32) * f32(SCALE)

    dfT = np.ascontiguousarray(dense_feat.transpose(0, 2, 1), dtype=f32)
    dxT = np.ascontiguousarray(dense_xyz.transpose(0, 2, 1), dtype=f32)
    sfT = np.ascontiguousarray(sparse_feat.transpose(0, 2, 1), dtype=f32)
    sxT = np.ascontiguousarray(sparse_xyz.transpose(0, 2, 1), dtype=f32)

    # ds = sum_d kaug[d] * qaug[d] = 0.25 * dist^2, computed as an fp16
    # matmul.  Naive [qn, 1, -2q] x [1, kn, k] augmentation cancels
    # catastrophically once inputs are rounded (negative ds -> sqrt NaN), so
    # every value is split hi/lo into two fp16 parts; fp16 x fp16 products
    # are exact in the fp32 PSUM accumulator, leaving ~3e-6 total error.
    f16, f64 = np.float16, np.float64

    def hilo(x):
        hi = x.astype(f16)
        lo = (x - hi.astype(f64)).astype(f16)
        return hi, lo

    qn = np.sum(dense_xyz.astype(f64) ** 2, axis=-1)   # [B, N2]
    kn = np.sum(sparse_xyz.astype(f64) ** 2, axis=-1)  # [B, N1]
    qnh, qnl = hilo(qn)
    knh, knl = hilo(kn)
    qch, qcl = hilo(dxT.astype(f64))                   # [B, 3, N2] each
    kch, kcl = hilo(sxT.astype(f64))
    one1 = np.ones((B, 1, N1), f16)
    quart2 = np.full((B, 1, N2), 0.25, f16)
    qaug = np.concatenate(
        [0.25 * qnh[:, None, :].astype(f16), 0.25 * qnl[:, None, :].astype(f16),
         quart2, quart2,
         -0.5 * qch, -0.5 * qch, -0.5 * qcl, -0.5 * qcl], axis=1).astype(f16)
    kaug = np.concatenate(
        [one1, one1, knh[:, None, :], knl[:, None, :],
         kch, kcl, kch, kcl], axis=1).astype(f16)

    common = {
        "sfT16": sfT.astype(np.float16),
        # negated xyz contribution of K plus bias (kernel accumulates -st)
        "kxT": np.ascontiguousarray(
            (-(sparse_xyz.astype(f64) @ Wk[:, :3].T.astype(f64)
               + bk.astype(f64)[None, None, :])).transpose(0, 2, 1)
        ).astype(np.float16),
        "kaug": kaug,
        "WqfT": np.ascontiguousarray(Wq[:, 3:].T, f32),
        "WqxT": np.ascontiguousarray(Wq[:, :3].T, f32),
        "WkfT": np.ascontiguousarray((-Wk[:, 3:]).T.astype(np.float16)),
        "WvT": np.ascontiguousarray(Wv.T.astype(np.float16)),
        "WoT": np.ascontiguousarray(Wo.T.astype(np.float16)),
        "bq": bq_s.reshape(FEAT, 1),
        "bv": bv.astype(f32).reshape(FEAT, 1),
        # bo' = Wo @ bv + bo (bv dropped from the xt add on device)
        "bo": (Wo.astype(f64) @ bv.astype(f64)
               + bo.astype(f64)).astype(f32).reshape(FEAT, 1),
    }
    in_maps = []
    for c in range(NCORES):
        sl = slice(c * QPC, (c + 1) * QPC)
        m = dict(common)
        m["dfT"] = np.ascontiguousarray(dfT[:, :, sl])
        m["dxT"] = np.ascontiguousarray(dxT[:, :, sl])
        m["qaug"] = np.ascontiguousarray(qaug[:, :, sl])
        in_maps.append(m)
    return in_maps


def run_sharded(in_maps, trace=False):
    nc = _build()
    kwargs = {}
    if trace:
        kwargs = {"trace": True}
    return bass_utils.run_bass_kernel_spmd(
        nc, in_maps, core_ids=list(range(NCORES)), **kwargs)


def kernel(sparse_xyz, sparse_feat, dense_xyz, dense_feat,
           Wq, bq, Wk, bk, Wv, bv, Wo, bo):
    args = [np.asarray(a) for a in (sparse_xyz, sparse_feat, dense_xyz,
                                    dense_feat, Wq, bq, Wk, bk, Wv, bv,
                                    Wo, bo)]
    in_maps = _prep_inputs(*args)
    res = run_sharded(in_maps, trace=bool(os.environ.get("BASS_KERNEL_TRACE")))
    out = np.empty((B, N2, FEAT), dtype=np.float32)
    for c in range(NCORES):
        out[:, c * QPC:(c + 1) * QPC, :] = \
            res.results[c]["outT"].transpose(0, 2, 1)
    if os.environ.get("BASS_KERNEL_TRACE"):
        print("HW exec time:", res.exec_time_ns, "ns")
    return out


# revision 27
# speedup vs baseline: 1.0032x; 1.0016x over previous
"""Trainium2 Bass kernel for nn_AttentionPropagation (sparse attention propagation).

Reference computation:
  Q = cat(dense_xyz, dense_feat) @ Wq.T + bq            [B, N2, F]
  K = cat(sparse_xyz, sparse_feat) @ Wk.T + bk          [B, N1, F]
  V = sparse_feat @ Wv.T + bv                           [B, N1, F]
  attn = softmax(Q K^T / sqrt(F) - 0.5 * dist(dense_xyz, sparse_xyz))
  out = (attn @ V + dense_feat) @ Wo.T + bo             [B, N2, F]

Shapes: B=2, N1=4096 (sparse/keys), N2=32768 (dense/queries), F=128.
Sharding: queries (N2) split across 8 cores; sparse K/V + weights replicated.

Per-core kernel, transposed layout (keys on partitions, queries on free dim).
Per pair of key chunks (2x128 keys x 512 queries, one [128,1024] PSUM tile):
  ds = 0.25*dist^2            (PE, K=16 fp16 hi/lo-split aug matmul; exact)
  tile = sqrt(ds + eps)       (ACT, in place on PSUM)
  tile += -st                 (PE accumulates -K@Q^T; K negated on host)
  spre = copy(tile) -> f16    (DVE; spre = 0.5*dist - st)
  attn = exp(-spre) -> fp8e4  (ACT, scale=-1)
  P^T += V8 @ attn            (PE, fp8 DoubleRow over chunk pairs; V in
                               single-level fp8e4 -- softmax renormalization
                               cancels most of the quantization)
  sums += ones @ attn         (PE, fp8 DoubleRow, all-ones stationary ->
                               every psum row holds the sums; no broadcast)
  out^T = Wo @ ((P^T * 1/sums) + dense_feat^T) + bo'   (bo' = Wo@bv + bo,
                               folded on host; the xt add runs on Pool)
The old standalone DVE subtract pass and the Pool partition_broadcast are
gone; sqrt and exp still alternate in table-set phases over units of 2-3
query groups to amortize the ~1.3us ACT table loads.  Phase A is
software-pipelined (ds of pair i+1 ahead of st of pair i) and each unit's
phase B pre-issues the next unit's DMAs, Q projection and first ds pairs.
"""

import os
import numpy as np

os.environ.setdefault("JAX_COMPILATION_CACHE_DIR", "/tmp/jax_bass_cache")
os.environ.setdefault("JAX_PERSISTENT_CACHE_MIN_ENTRY_SIZE_BYTES", "0")
os.environ.setdefault("JAX_PERSISTENT_CACHE_MIN_COMPILE_TIME_SECS", "1")

import concourse.bacc as bacc
import concourse.tile as tile
import concourse.mybir as mybir
from concourse import bass_utils
from concourse.tile import add_dep_helper

F32 = mybir.dt.float32
F32R = mybir.dt.float32r
F16 = mybir.dt.float16
F8 = mybir.dt.float8e4
AF = mybir.ActivationFunctionType
OP = mybir.AluOpType
DR = mybir.MatmulPerfMode.DoubleRow

B = 2
N1 = 4096          # sparse points (keys)
N2 = 32768         # dense points (queries)
FEAT = 128
SCALE = FEAT ** -0.5
NCORES = 8
QPC = N2 // NCORES  # queries per core per batch (4096)
QG = 512            # query group (matmul moving free dim)
GROUPS = QPC // QG  # 8 groups per batch
KC = 128            # key chunk (PSUM partition dim)
CHUNKS = N1 // KC   # 32
PAIRS = CHUNKS // 2  # 16 chunk pairs (one [128,1024] psum tile each)
UNITS = [(0, 1), (2, 3, 4), (5, 6, 7)]  # groups per sqrt/exp table unit
NAUG = 16           # hi/lo-split dist^2 augmentation rows
SQRT_EPS = 1e-5     # covers residual fp16-split ds error (~3e-6) and the
                    # reference's 1e-12 clamp; adds <2e-3 logit error

_NC_CACHE = {}


def _r2(ap):
    # [p, (two f)] -> [p, two, f] for DoubleRow matmuls
    return ap.rearrange("p (two f) -> p two f", two=2)


def _build():
    if "nc" in _NC_CACHE:
        return _NC_CACHE["nc"]
    nc = bacc.Bacc("TRN2", target_bir_lowering=False, debug=False)

    # ---- DRAM I/O (per-core shard) ----
    dfT = nc.dram_tensor("dfT", [B, FEAT, QPC], F32R, kind="ExternalInput")
    dxT = nc.dram_tensor("dxT", [B, 3, QPC], F32R, kind="ExternalInput")
    qaug = nc.dram_tensor("qaug", [B, NAUG, QPC], F16, kind="ExternalInput")
    sfT16 = nc.dram_tensor("sfT16", [B, FEAT, N1], F16, kind="ExternalInput")
    kxT = nc.dram_tensor("kxT", [B, FEAT, N1], F16, kind="ExternalInput")  # -(xyz@WkxT+bk)
    kaug = nc.dram_tensor("kaug", [B, NAUG, N1], F16, kind="ExternalInput")
    WqfT = nc.dram_tensor("WqfT", [FEAT, FEAT], F32R, kind="ExternalInput")
    WqxT = nc.dram_tensor("WqxT", [3, FEAT], F32R, kind="ExternalInput")
    WkfT = nc.dram_tensor("WkfT", [FEAT, FEAT], F16, kind="ExternalInput")  # negated
    WvT = nc.dram_tensor("WvT", [FEAT, FEAT], F16, kind="ExternalInput")
    WoT = nc.dram_tensor("WoT", [FEAT, FEAT], F16, kind="ExternalInput")
    bq = nc.dram_tensor("bq", [FEAT, 1], F32, kind="ExternalInput")
    bv = nc.dram_tensor("bv", [FEAT, 1], F32, kind="ExternalInput")
    bo = nc.dram_tensor("bo", [FEAT, 1], F32, kind="ExternalInput")
    outT = nc.dram_tensor("outT", [B, FEAT, QPC], F32, kind="ExternalOutput")

    with tile.TileContext(nc) as tc:
        with tc.tile_pool(name="const", bufs=1) as const_p, \
             tc.tile_pool(name="batch", bufs=1) as batch_p, \
             tc.tile_pool(name="slab", bufs=3) as slab_p, \
             tc.tile_pool(name="attn", bufs=2) as attn_p, \
             tc.tile_pool(name="stage", bufs=1) as stage_p, \
             tc.tile_pool(name="gsmall", bufs=4) as gsm_p, \
             tc.tile_pool(name="gout", bufs=2) as gout_p, \
             tc.tile_pool(name="ps_pair", bufs=3, space="PSUM") as ps_pair, \
             tc.tile_pool(name="ps_pt", bufs=1, space="PSUM") as ps_pt, \
             tc.tile_pool(name="ps_sm", bufs=1, space="PSUM") as ps_sm:

            # ---- constants ----
            wqf_t = const_p.tile([FEAT, FEAT], F32R)
            wqx_t = const_p.tile([3, FEAT], F32R)
            wkf_t = const_p.tile([FEAT, FEAT], F16)
            wv_t = const_p.tile([FEAT, FEAT], F16)
            wo_t = const_p.tile([FEAT, FEAT], F16)
            bq_t = const_p.tile([FEAT, 1], F32)
            bv_t = const_p.tile([FEAT, 1], F32)
            bo_t = const_p.tile([FEAT, 1], F32)
            ones8 = const_p.tile([KC, 2 * KC], F8)
            eps_t = const_p.tile([KC, 1], F32)
            dummy_t = const_p.tile([1, 1], F32)
            nc.vector.memset(eps_t, SQRT_EPS)
            # preload the sqrt table at t=0, overlapping the input DMAs
            nc.scalar.activation(dummy_t, eps_t[0:1, 0:1], AF.Sqrt,
                                 bias=eps_t[0:1, 0:1])
            for t, d in ((wqf_t, WqfT), (wqx_t, WqxT), (wkf_t, WkfT),
                         (wv_t, WvT), (wo_t, WoT),
                         (bq_t, bq), (bo_t, bo)):
                # constants ride the SWDGE queue to keep the HWDGE queue free
                # for the first group's inputs at startup
                nc.gpsimd.dma_start(out=t, in_=d.ap())
            nc.vector.memset(ones8, 1.0)

            # ACT runs exactly two table-based funcs (Sqrt, Exp) in different
            # table sets; a table switch costs ~1.3us.  Pin every sqrt of
            # unit u+1 behind the last exp of unit u so the scheduler cannot
            # interleave the phases.
            last_exp = [None]
            last_sqrt = [None]

            bst = {}      # batch -> dict of per-batch slabs
            pre_dma = {}  # (b, g) -> (df_t, dx_t, qa_t)
            pre_qt = {}   # (b, g) -> qt tile
            pre_ds = {}   # (b, g, p) -> pair psum tile (ds done, sqrt pending)

            def batch_dmas(b):
                st = {}
                st["sf16"] = stage_p.tile([FEAT, N1], F16, tag="sf16", name="sf16")
                st["kx"] = stage_p.tile([FEAT, N1], F16, tag="kx", name="kx")
                st["ka"] = batch_p.tile([NAUG, N1], F16, tag="ka", name="ka")
                nc.sync.dma_start(out=st["ka"], in_=kaug.ap()[b])
                bst[b] = st
                return st

            def batch_dmas2(b):
                st = bst[b]
                nc.sync.dma_start(out=st["kx"], in_=kxT.ap()[b])
                # chunked so the j=0 K^T projection starts on the first slice
                for j in range(N1 // QG):
                    nc.sync.dma_start(
                        out=st["sf16"][:, j * QG:(j + 1) * QG],
                        in_=sfT16.ap()[b, :, j * QG:(j + 1) * QG])

            def kproj_alloc(b):
                st = bst[b]
                st["kt"] = batch_p.tile([FEAT, N1], F16, tag="kt", name="kt")  # -K^T
                st["vhi"] = batch_p.tile([KC, CHUNKS * FEAT], F8, tag="vhi", name="vhi")
                st["kproj_j"] = 0

            def kproj_step(b, upto_j):
                st = bst[b]
                while st["kproj_j"] <= min(upto_j, N1 // QG - 1):
                    j = st["kproj_j"]
                    ps = ps_pair.tile([KC, 2 * QG], F32, tag="pair")
                    nc.tensor.matmul(ps[:, 0:QG], wkf_t,
                                     st["sf16"][:, j * QG:(j + 1) * QG],
                                     start=True, stop=True)
                    # xyz part of -K plus bias, folded on the host into kxT
                    nc.vector.tensor_tensor(
                        out=st["kt"][:, j * QG:(j + 1) * QG], in0=ps[:, 0:QG],
                        in1=st["kx"][:, j * QG:(j + 1) * QG], op=OP.add)
                    st["kproj_j"] = j + 1

            def batch_kproj(b):
                kproj_alloc(b)
                kproj_step(b, N1 // QG - 1)

            def project_v(b):
                # V is only read in phase B; emitting it after the first
                # unit's phase A keeps PE on the critical path early on.
                # Single-level fp8 V: softmax renormalization cancels most
                # of the quantization (golden model: +1e-4 rel err).
                st = bst[b]
                for c in range(CHUNKS):
                    ps = ps_pt.tile([KC, QG], F32, tag="pt")
                    nc.tensor.matmul(ps[:, 0:FEAT],
                                     st["sf16"][:, c * KC:(c + 1) * KC], wv_t,
                                     start=True, stop=True)
                    nc.vector.tensor_copy(
                        st["vhi"][:, c * FEAT:(c + 1) * FEAT], ps[:, 0:FEAT])

            def group_dmas(b, g):
                q0 = g * QG
                df_t = gsm_p.tile([FEAT, QG], F32R, tag="df", bufs=4)
                dx_t = gsm_p.tile([3, QG], F32R, tag="dx")
                qa_t = gsm_p.tile([NAUG, QG], F16, tag="qa")
                # qa first: it gates the ds matmuls -> sqrt pipeline
                nc.sync.dma_start(out=qa_t, in_=qaug.ap()[b, :, q0:q0 + QG])
                nc.sync.dma_start(out=dx_t, in_=dxT.ap()[b, :, q0:q0 + QG])
                nc.sync.dma_start(out=df_t, in_=dfT.ap()[b, :, q0:q0 + QG])
                r = (df_t, dx_t, qa_t)
                pre_dma[(b, g)] = r
                return r

            def do_qt(b, g):
                df_t, dx_t, qa_t = pre_dma[(b, g)]
                ps_q = ps_pair.tile([KC, 2 * QG], F32, tag="pair")
                nc.tensor.matmul(ps_q[:, 0:QG], wqf_t, df_t,
                                 start=True, stop=False)
                nc.tensor.matmul(ps_q[:, 0:QG], wqx_t, dx_t,
                                 start=False, stop=True)
                qt_t = gsm_p.tile([FEAT, QG], F16, tag="qt")
                nc.vector.tensor_scalar_add(qt_t, ps_q[:, 0:QG], bq_t)
                pre_qt[(b, g)] = qt_t
                return qt_t

            def pair_ds(b, g, p):
                """ds matmuls into a fresh pair tile (PE only)."""
                qa_t = pre_dma[(b, g)][2]
                ka_t = bst[b]["ka"]
                pt_ps = ps_pair.tile([KC, 2 * QG], F32, tag="pair")
                for ci, c in ((0, 2 * p), (1, 2 * p + 1)):
                    nc.tensor.matmul(pt_ps[:, ci * QG:(ci + 1) * QG],
                                     ka_t[:, c * KC:(c + 1) * KC],
                                     qa_t, start=True, stop=True)
                pre_ds[(b, g, p)] = pt_ps
                return pt_ps

            def pair_sqrt(pt_ps):
                sq_i = nc.scalar.activation(pt_ps, pt_ps, AF.Sqrt,
                                            bias=eps_t[:, 0:1])
                if last_exp[0] is not None:
                    add_dep_helper(sq_i.ins, last_exp[0],
                                   reason="ACT table phase order")
                last_sqrt[0] = sq_i.ins

            def pair_back(b, g, p, pt_ps, sp):
                """-st accumulate onto the sqrt'd tile + f16 copy out."""
                qt_t = pre_qt[(b, g)]
                kt_t = bst[b]["kt"]
                c0 = 2 * p
                for ci, c in ((0, c0), (1, c0 + 1)):
                    nc.tensor.matmul(pt_ps[:, ci * QG:(ci + 1) * QG],
                                     kt_t[:, c * KC:(c + 1) * KC], qt_t,
                                     start=False, stop=True,
                                     skip_group_check=True)
                nc.vector.tensor_copy(sp[:, c0 * QG:(c0 + 2) * QG], pt_ps)

            units_all = [(b, gs) for b in range(B) for gs in UNITS]
            NPRE = 2  # pair tiles pre-ds'ed for the next unit during phase B

            # ---- startup: first group's DMAs + Q proj, then batch-0 setup ----
            batch_dmas(0)
            group_dmas(0, 0)
            do_qt(0, 0)
            batch_dmas2(0)
            batch_kproj(0)

            for ui, (b, gs) in enumerate(units_all):
                first_of_batch = (gs is UNITS[0])
                nxt = units_all[ui + 1] if ui + 1 < len(units_all) else None

                # ---------- phase A: ds -> sqrt -> -st -> f16 copy ----------
                # Software-pipelined: the ds+sqrt of pair i+1 is emitted
                # before the st matmuls of pair i, so the in-order PE queue
                # never waits on the ACT sqrt.
                spre = {}
                work = []  # (b, g, p, pt_ps, sp) awaiting back-half
                for g in gs:
                    if (b, g) not in pre_dma:
                        group_dmas(b, g)
                    if (b, g) not in pre_qt:
                        do_qt(b, g)
                    sp = slab_p.tile([KC, CHUNKS * QG], F16, tag="spre")
                    spre[g] = sp
                    for p in range(PAIRS):
                        pt_ps = pre_ds.pop((b, g, p), None)
                        if pt_ps is None:
                            pt_ps = pair_ds(b, g, p)
                        pair_sqrt(pt_ps)
                        work.append((b, g, p, pt_ps, sp))
                        if len(work) > 1:
                            pair_back(*work.pop(0))
                while work:
                    pair_back(*work.pop(0))

                if first_of_batch:
                    project_v(b)

                # ---------- phase B: exp -> attn@V (fp8 DR) -> out ----------
                for gi, g in enumerate(gs):
                    q0 = g * QG
                    sp = spre[g]
                    ap8 = attn_p.tile([KC, CHUNKS * QG], F8, tag="ap8")
                    pt = ps_pt.tile([KC, QG], F32, tag="pt")
                    sm = ps_sm.tile([KC, QG], F32, tag="sm")
                    # finer exp slices for the very last group shorten the tail
                    n_exp = 8 if nxt is None and g == gs[-1] else 2
                    per = PAIRS // n_exp
                    for e in range(n_exp):
                        lo, hi = e * per * 2 * QG, (e + 1) * per * 2 * QG
                        exp_i = nc.scalar.activation(
                            ap8[:, lo:hi], sp[:, lo:hi], AF.Exp, scale=-1.0)
                        if last_sqrt[0] is not None:
                            add_dep_helper(exp_i.ins, last_sqrt[0],
                                           reason="ACT table phase order")
                        last_exp[0] = exp_i.ins
                        for p in range(e * per, (e + 1) * per):
                            c0 = 2 * p
                            at_r = _r2(ap8[:, c0 * QG:(c0 + 2) * QG])
                            first = (p == 0)
                            last = (p == PAIRS - 1)
                            nc.tensor.matmul(
                                pt[0:FEAT, :],
                                _r2(bst[b]["vhi"][:, c0 * FEAT:(c0 + 2) * FEAT]),
                                at_r, start=first, stop=last, perf_mode=DR)
                            nc.tensor.matmul(
                                sm, _r2(ones8), at_r,
                                start=first, stop=last, perf_mode=DR)
                    last_grp = nxt is None and g == gs[-1]
                    if not last_grp:
                        # copy P^T out of PSUM so the single pt bank is free
                        # for the next group's DoubleRow accumulation
                        xc_t = gout_p.tile([FEAT, QG], F16, tag="xc")
                        nc.vector.tensor_copy(xc_t, pt[0:FEAT, :])
                        pt_src = xc_t
                    else:
                        pt_src = pt[0:FEAT, :]  # skip the copy on the tail
                    # every row of sm holds the key-sums for its query
                    rsm_t = gout_p.tile([KC, QG], F32, tag="rsm")
                    x1_t = gout_p.tile([FEAT, QG], F32, tag="x1")
                    xt_t = gout_p.tile([FEAT, QG], F16, tag="xt")
                    po = ps_sm.tile([KC, QG], F32, tag="sm")
                    o_t = gout_p.tile([FEAT, QG], F32, tag="o")
                    # the last group's chain is the kernel tail: run it in two
                    # column halves so the five stages pipeline
                    halves = ((0, QG // 2), (QG // 2, QG)) if last_grp \
                        else ((0, QG),)
                    for lo, hi in halves:
                        nc.vector.reciprocal(rsm_t[:, lo:hi], sm[:, lo:hi])
                        nc.vector.tensor_tensor(
                            out=x1_t[:, lo:hi], in0=pt_src[:, lo:hi],
                            in1=rsm_t[:, lo:hi], op=OP.mult)
                        # bv folded into bo on the host (bo' = Wo@bv + bo), so
                        # xt is a plain add -- runs on the otherwise idle Pool
                        nc.gpsimd.tensor_tensor(
                            out=xt_t[:, lo:hi], in0=x1_t[:, lo:hi],
                            in1=pre_dma[(b, g)][0].bitcast(F32)[:, lo:hi],
                            op=OP.add)
                        nc.tensor.matmul(po[0:FEAT, lo:hi], wo_t,
                                         xt_t[:, lo:hi], start=True, stop=True)
                        nc.vector.tensor_scalar_add(
                            o_t[:, lo:hi], po[0:FEAT, lo:hi], bo_t)
                        nc.sync.dma_start(
                            out=outT.ap()[b, :, q0 + lo:q0 + hi],
                            in_=o_t[:, lo:hi])

                    # -- pre-work for the next unit, spread through phase B --
                    if nxt is not None:
                        nb, ngs = nxt
                        if gi == 0:
                            # DMA starts only (SP queue; lands during B)
                            if nb != b:
                                batch_dmas(nb)
                            group_dmas(nb, ngs[0])
                            if nb != b:
                                batch_dmas2(nb)
                        elif gi == 1:
                            # PE/DVE pre-work after the first group's B block
                            do_qt(nb, ngs[0])
                            for p in range(NPRE):
                                pair_ds(nb, ngs[0], p)
                        if nb != b and gi == len(gs) - 1:
                            # K projection last: its kx/sf16 DMAs need time
                            batch_kproj(nb)

    nc.compile()
    _NC_CACHE["nc"] = nc
    return nc


def _prep_inputs(sparse_xyz, sparse_feat, dense_xyz, dense_feat,
                 Wq, bq, Wk, bk, Wv, bv, Wo, bo):
    """Host-side layout prep: transposes, weight folding, xyz augmentation."""
    f32 = np.float32
    Wq = Wq.astype(f32) * f32(SCALE)
    bq_s = bq.astype(f32) * f32(SCALE)

    dfT = np.ascontiguousarray(dense_feat.transpose(0, 2, 1), dtype=f32)
    dxT = np.ascontiguousarray(dense_xyz.transpose(0, 2, 1), dtype=f32)
    sfT = np.ascontiguousarray(sparse_feat.transpose(0, 2, 1), dtype=f32)
    sxT = np.ascontiguousarray(sparse_xyz.transpose(0, 2, 1), dtype=f32)

    # ds = sum_d kaug[d] * qaug[d] = 0.25 * dist^2, computed as an fp16
    # matmul.  Naive [qn, 1, -2q] x [1, kn, k] augmentation cancels
    # catastrophically once inputs are rounded (negative ds -> sqrt NaN), so
    # every value is split hi/lo into two fp16 parts; fp16 x fp16 products
    # are exact in the fp32 PSUM accumulator, leaving ~3e-6 total error.
    f16, f64 = np.float16, np.float64

    def hilo(x):
        hi = x.astype(f16)
        lo = (x - hi.astype(f64)).astype(f16)
        return hi, lo

    qn = np.sum(dense_xyz.astype(f64) ** 2, axis=-1)   # [B, N2]
    kn = np.sum(sparse_xyz.astype(f64) ** 2, axis=-1)  # [B, N1]
    qnh, qnl = hilo(qn)
    knh, knl = hilo(kn)
    qch, qcl = hilo(dxT.astype(f64))                   # [B, 3, N2] each
    kch, kcl = hilo(sxT.astype(f64))
    one1 = np.ones((B, 1, N1), f16)
    quart2 = np.full((B, 1, N2), 0.25, f16)
    qaug = np.concatenate(
        [0.25 * qnh[:, None, :].astype(f16), 0.25 * qnl[:, None, :].astype(f16),
         quart2, quart2,
         -0.5 * qch, -0.5 * qch, -0.5 * qcl, -0.5 * qcl], axis=1).astype(f16)
    kaug = np.concatenate(
        [one1, one1, knh[:, None, :], knl[:, None, :],
         kch, kcl, kch, kcl], axis=1).astype(f16)

    common = {
        "sfT16": sfT.astype(np.float16),
        # negated xyz contribution of K plus bias (kernel accumulates -st)
        "kxT": np.ascontiguousarray(
            (-(sparse_xyz.astype(f64) @ Wk[:, :3].T.astype(f64)
               + bk.astype(f64)[None, None, :])).transpose(0, 2, 1)
        ).astype(np.float16),
        "kaug": kaug,
        "WqfT": np.ascontiguousarray(Wq[:, 3:].T, f32),
        "WqxT": np.ascontiguousarray(Wq[:, :3].T, f32),
        "WkfT": np.ascontiguousarray((-Wk[:, 3:]).T.astype(np.float16)),
        "WvT": np.ascontiguousarray(Wv.T.astype(np.float16)),
        "WoT": np.ascontiguousarray(Wo.T.astype(np.float16)),
        "bq": bq_s.reshape(FEAT, 1),
        "bv": bv.astype(f32).reshape(FEAT, 1),
        # bo' = Wo @ bv + bo (bv dropped from the xt add on device)
        "bo": (Wo.astype(f64) @ bv.astype(f64)
               + bo.astype(f64)).astype(f32).reshape(FEAT, 1),
    }
    in_maps = []
    for c in range(NCORES):
        sl = slice(c * QPC, (c + 1) * QPC)
        m = dict(common)
        m["dfT"] = np.ascontiguousarray(dfT[:, :, sl])
        m["dxT"] = np.ascontiguousarray(dxT[:, :, sl])
        m["qaug"] = np.ascontiguousarray(qaug[:, :, sl])
        in_maps.append(m)
    return in_maps


def run_sharded(in_maps, trace=False):
    nc = _build()
    kwargs = {}
    if trace:
        kwargs = {"trace": True}
    return bass_utils.run_bass_kernel_spmd(
        nc, in_maps, core_ids=list(range(NCORES)), **kwargs)


def kernel(sparse_xyz, sparse_feat, dense_xyz, dense_feat,
           Wq, bq, Wk, bk, Wv, bv, Wo, bo):
    args = [np.asarray(a) for a in (sparse_xyz, sparse_feat, dense_xyz,
                                    dense_feat, Wq, bq, Wk, bk, Wv, bv,
                                    Wo, bo)]
    in_maps = _prep_inputs(*args)
    res = run_sharded(in_maps, trace=bool(os.environ.get("BASS_KERNEL_TRACE")))
    out = np.empty((B, N2, FEAT), dtype=np.float32)
    for c in range(NCORES):
        out[:, c * QPC:(c + 1) * QPC, :] = \
            res.results[c]["outT"].transpose(0, 2, 1)
    if os.environ.get("BASS_KERNEL_TRACE"):
        print("HW exec time:", res.exec_time_ns, "ns")
    return out
